# revision 5
# baseline (speedup 1.0000x reference)
"""NT-Xent / contrastive loss on 8 Trainium2 NeuronCores.

Reference computation (B=4096, D=512, temp=0.1):
    z   = l2norm(concat(proj_1, proj_2))          # [8192, 512]
    cos = (z @ z.T) / temp                        # [8192, 8192]
    pos[r]  = cos[r, (r + 4096) % 8192]
    lse[r]  = logsumexp(cos[r, :] with cos[r, r] masked out)
    loss    = mean(lse - pos)

Sharding: similarity-matrix rows, 1024 per core; inputs host-rotated by
core*1024 rows so the SPMD program is uniform (self-diagonal at local
col == row, positive at local col == row + 4096).

v2.5 design (vs 238us bf16 baseline, 169us fp8 v2):
  - Each core normalizes/transposes ONLY its own 1024 rows: dense f32
    pair loads, square+rowsum (DVE stt), fast-rsqrt (DVE int magic + 2
    Newton steps), normalize+bf16 downcast (ts_mul by per-partition
    16/||x||), PE transpose vs bf16 identity, PSUM->SBUF fp8 e4m3
    evacuation.  The fp8 transposed block zq.T = (16*l2norm(x)).T
    [128, 4, 1024] is AllGathered across the 8 cores via HBM (512KB per
    core), then 7 dynamic-offset DMAs (partition_id register) re-rotate
    the gathered ranks into this core's local column order.  This
    removes 7/8 of the phase-1 vector work and input DMA vs v2.
  - GEMM in fp8 with MatmulPerfMode.DoubleRow (2 K-subtiles per
    instruction) = 2x bf16 PE throughput; psum = 256*cos, folded out
    via the Exp scale 10/256.  The J=0 column group (own rows, needs no
    collective) is emitted first so the PE works during the AllGather.
  - Row logsumexp: ScalarE Exp (in-place PSUM, accum_out row sums) for
    most J-groups; a tuned subset uses a DVE Schraudolph exp (int32
    bits = A*x + B, zero-mean constant) + reduce_sum to balance the
    exp-bound ScalarE against the otherwise-idle DVE.  Self/positive
    diagonals are pulled from raw PSUM (multiply-by-identity with
    accum) before the in-place Exp; the self term is subtracted via the
    identical ScalarE Exp pathway so it cancels exactly.
Host adds the 8 partial sums and divides by 8192.
"""

import sys

import numpy as np

if "/opt/trn_rl_repo" not in sys.path:
    sys.path.insert(0, "/opt/trn_rl_repo")

_B = 4096
_D = 512
_N2 = 2 * _B            # 8192 rows of the similarity matrix
_NCORES = 8
_RPC = _N2 // _NCORES   # 1024 rows per core
_INV_TEMP = 10.0
_FSC = 16.0             # fp8 scale: zq = 16*z, psum = 256*cos
_ESC = _INV_TEMP / (_FSC * _FSC)   # Exp scale applied to psum

_GRP = 8                # own row tiles per core
_NM = _RPC // 128       # 8 output row blocks
_NG = _N2 // 1024       # 8 column groups (one per source rank)
_NK = _D // 128         # 4 contraction chunks (2 DoubleRow pairs)

_MAGIC1 = 0x5F3759E0    # fast inverse sqrt magic + 1 (M - x == (M+1) + ~x)

# Schraudolph exp on psum: float32 bits = A*x + B (zero-mean constant)
_SCH_C = 0.05640058203329989
_SCH_A = float((2.0 ** 23) / np.log(2.0) * _ESC)
_SCH_B = float(127.0 * 2 ** 23 - _SCH_C * 2 ** 23)


def _sch_group(m, g):
    # J-groups routed to the DVE Schraudolph exp; groups 0 (self diag)
    # and 4 (positive diag) must stay on ScalarE.
    return g in (5, 6) or (g == 3 and m % 2 == 0)


def _emit(tc, projs, out_partial):
    import concourse.bass as bass  # noqa: F401
    from concourse.bass import ds
    from concourse import mybir

    nc = tc.nc
    f32 = mybir.dt.float32
    bf16 = mybir.dt.bfloat16
    fp8 = mybir.dt.float8e4
    i32 = mybir.dt.int32
    Alu = mybir.AluOpType
    Act = mybir.ActivationFunctionType
    DR = mybir.MatmulPerfMode.DoubleRow

    from contextlib import ExitStack
    ctx = ExitStack()
    pool = ctx.enter_context(tc.tile_pool(name="work", bufs=1))
    pers = ctx.enter_context(tc.tile_pool(name="pers", bufs=1))
    pspool = ctx.enter_context(tc.tile_pool(name="psum", bufs=1, space="PSUM"))
    dram = ctx.enter_context(tc.tile_pool(name="dram", bufs=1, space="DRAM"))

    # ---- constants ----
    ones = pers.tile([128, 128], f32, tag="ones")
    nc.vector.memset(ones[:], 1.0)
    ident = pers.tile([128, 128], f32, tag="ident")
    nc.gpsimd.affine_select(ident[:], ones[:], pattern=[[1, 128]],
                            compare_op=Alu.is_equal, fill=0.0,
                            base=0, channel_multiplier=-1)
    identb = pers.tile([128, 128], bf16, tag="identb")
    nc.vector.tensor_copy(identb[:], ident[:])

    # ---- persistent buffers ----
    # own transposed fp8 block: [p, k, own-row c] (c in 0..1023)
    zown = pers.tile([128, _NK * _RPC], fp8, tag="zown")
    zo3 = zown.rearrange("p (k c) -> p k c", k=_NK)
    # gathered/rotated transposed z, local column order [p, k, 0..8191]
    zt = pers.tile([128, _NK * _N2], fp8, tag="zt")
    zt3 = zt.rearrange("p (k c) -> p k c", k=_NK)
    sp_all = pers.tile([128, 2 * _NM], f32, tag="sp")    # self | pos diags
    se_all = pers.tile([128, _NM * _NG], f32, tag="se")  # per (m, J) sums

    pv = projs.rearrange("(u two p) d -> u p two d", two=2, p=128)

    # ---- phase 1: own 8 tiles only ----
    raw2s = []
    ss = pool.tile([128, _GRP], f32, tag="ss")
    for u in range(_GRP // 2):
        raw2 = pool.tile([128, 2, _D], f32, tag="raw", bufs=4, name=f"raw{u}")
        nc.sync.dma_start(raw2[:], pv[u])
        raw2s.append(raw2)
        for h in range(2):
            i = 2 * u + h
            sq = pool.tile([128, _D], bf16, tag="sq", bufs=2, name=f"sq{i}")
            nc.vector.scalar_tensor_tensor(
                out=sq[:], in0=raw2[:, h], scalar=1.0, in1=raw2[:, h],
                op0=Alu.mult, op1=Alu.mult, accum_out=ss[:, i:i + 1])

    # rn16 = 16/sqrt(max(ss,1e-24)): fast-rsqrt + 2 Newton steps
    ssc = pool.tile([128, _GRP], f32, tag="ssc")
    nc.vector.tensor_scalar_max(ssc[:], ss[:], 1e-24)
    ti = pool.tile([128, _GRP], i32, tag="ti")
    nc.vector.tensor_scalar(
        out=ti[:], in0=ssc[:].bitcast(i32), scalar1=1, scalar2=-1,
        op0=Alu.logical_shift_right, op1=Alu.bitwise_xor)
    rn = pool.tile([128, _GRP], f32, tag="rn")
    nc.vector.tensor_scalar(
        out=rn[:].bitcast(i32), in0=ti[:], scalar1=_MAGIC1, scalar2=None,
        op0=Alu.add)
    nt = pool.tile([128, _GRP], f32, tag="nt")
    for _ in range(2):
        nc.vector.tensor_tensor(out=nt[:], in0=rn[:], in1=rn[:], op=Alu.mult)
        nc.vector.tensor_tensor(out=nt[:], in0=nt[:], in1=ssc[:], op=Alu.mult)
        nc.vector.tensor_scalar(out=nt[:], in0=nt[:], scalar1=-0.5,
                                scalar2=1.5, op0=Alu.mult, op1=Alu.add)
        nc.vector.tensor_tensor(out=rn[:], in0=rn[:], in1=nt[:], op=Alu.mult)
    rn16 = pool.tile([128, _GRP], f32, tag="rn16")
    nc.vector.tensor_scalar(out=rn16[:], in0=rn[:], scalar1=_FSC,
                            scalar2=None, op0=Alu.mult)

    for i in range(_GRP):
        rawb = pool.tile([128, _D], bf16, tag="rawb", bufs=4, name=f"rawb{i}")
        nc.vector.tensor_scalar_mul(rawb[:], raw2s[i // 2][:, i % 2],
                                    rn16[:, i:i + 1])
        psT = pspool.tile([128, _D], f32, tag="psT", bufs=2, name=f"psT{i}")
        for d in range(_NK):
            nc.tensor.matmul(psT[:, d * 128:(d + 1) * 128],
                             rawb[:, d * 128:(d + 1) * 128],
                             identb[:], start=True, stop=True)
        dst = zo3[:, :, i * 128:(i + 1) * 128]
        srcT = psT[:].rearrange("p (k c) -> p k c", k=_NK)
        if i % 2 == 0:
            nc.vector.tensor_copy(dst, srcT)
        else:
            nc.scalar.activation(dst, srcT, Act.Copy, bias=0.0, scale=1.0)

    # ---- AllGather own fp8 block, then re-rotate into local order ----
    gin = dram.tile([128, _NK * _RPC], fp8)
    gout = dram.tile([_NCORES * 128, _NK * _RPC], fp8)
    nc.sync.dma_start(gin[:], zown[:])
    nc.gpsimd.collective_compute(
        "AllGather", Alu.bypass, replica_groups=[list(range(_NCORES))],
        ins=[gin[:].opt()], outs=[gout[:].opt()])
    go4 = gout[:].rearrange("(r p) (k c) -> r p k c", p=128, k=_NK)

    cid = nc.sync.partition_id()
    for g in range(1, _NG):
        src_r = nc.sync.alloc_register(f"srcr{g}")
        nc.sync.reg_alu(src_r, cid, g, Alu.add)
        nc.sync.reg_alu(src_r, src_r, _NCORES, Alu.mod)
        src = nc.sync.snap(src_r, min_val=0, max_val=_NCORES - 1)
        nc.sync.dma_start(zt3[:, :, g * _RPC:(g + 1) * _RPC],
                          go4[ds(src, 1), :, :, :])

    # ---- phase 2: DoubleRow fp8 GEMM + exp row sums ----
    def gemm_expcol(m, g):
        rhs3 = zo3 if g == 0 else zt3
        ps = pspool.tile([128, 1024], f32, tag="ps", bufs=3,
                         name=f"ps{m}_{g}")
        for c in range(2):
            jlo = g * 1024 + c * 512
            for k2 in range(2):
                nc.tensor.matmul(
                    ps[:, c * 512:(c + 1) * 512],
                    zo3[:, 2 * k2:2 * k2 + 2, m * 128:(m + 1) * 128],
                    rhs3[:, 2 * k2:2 * k2 + 2, jlo:jlo + 512],
                    start=(k2 == 0), stop=(k2 == 1), perf_mode=DR)
        off = m * 128
        if g == 0 or g == _NG // 2:
            col = m if g == 0 else _NM + m
            junk = pool.tile([128, 128], f32, tag="junk", bufs=2,
                             name=f"junk{m}_{g}")
            nc.vector.scalar_tensor_tensor(
                out=junk[:], in0=ps[:, off:off + 128], scalar=1.0,
                in1=ident[:], op0=Alu.mult, op1=Alu.mult,
                accum_out=sp_all[:, col:col + 1])
        secol = se_all[:, m * _NG + g:m * _NG + g + 1]
        if _sch_group(m, g):
            tmp = pool.tile([128, 1024], i32, tag="tmp", bufs=2,
                            name=f"tmp{m}_{g}")
            nc.vector.tensor_scalar(
                out=tmp[:], in0=ps[:], scalar1=_SCH_A, scalar2=_SCH_B,
                op0=Alu.mult, op1=Alu.add)
            nc.vector.reduce_sum(out=secol, in_=tmp[:].bitcast(f32),
                                 axis=mybir.AxisListType.X)
        else:
            nc.scalar.activation(ps[:], ps[:], Act.Exp, bias=0.0,
                                 scale=_ESC, accum_out=secol)

    # J=0 first (no collective dependency) so PE overlaps the AllGather
    for g in range(_NG):
        for m in range(_NM):
            gemm_expcol(m, g)

    # ---- phase 3: lse, loss, partial sum ----
    rs = pool.tile([128, _NM], f32, tag="rs")
    se4 = se_all.rearrange("p (m j) -> p m j", m=_NM)
    nc.vector.tensor_reduce(out=rs[:], in_=se4, axis=mybir.AxisListType.X,
                            op=Alu.add)
    sx = pool.tile([128, _NM], f32, tag="sx")
    nc.scalar.activation(sx[:], sp_all[:, 0:_NM], Act.Exp, bias=0.0,
                         scale=_ESC)
    nc.vector.tensor_sub(rs[:], rs[:], sx[:])
    lse = pool.tile([128, _NM], f32, tag="lse")
    nc.scalar.activation(lse[:], rs[:], Act.Ln, bias=0.0, scale=1.0)
    loss = pool.tile([128, _NM], f32, tag="loss")
    nc.vector.scalar_tensor_tensor(
        out=loss[:], in0=sp_all[:, _NM:2 * _NM], scalar=-_ESC,
        in1=lse[:], op0=Alu.mult, op1=Alu.add)
    lossv = pool.tile([128, 1], f32, tag="lossv")
    nc.vector.reduce_sum(out=lossv[:], in_=loss[:], axis=mybir.AxisListType.X)
    pf = pspool.tile([1, 1], f32, tag="psT", bufs=2)
    nc.tensor.matmul(pf[:], lossv[:], ones[:, 0:1], start=True, stop=True)
    res = pool.tile([1, 1], f32, tag="res")
    nc.vector.tensor_copy(res[:], pf[:])
    nc.sync.dma_start(out_partial[:, :], res[:])

    ctx.close()


def build():
    import concourse.tile as tile
    from concourse import bacc, mybir

    nc = bacc.Bacc("TRN2", target_bir_lowering=False, debug=False,
                   enable_asserts=True, num_devices=_NCORES)
    projs = nc.dram_tensor("projs", [_RPC, _D], mybir.dt.float32,
                           kind="ExternalInput").ap()
    out_partial = nc.dram_tensor("partial", [1, 1], mybir.dt.float32,
                                 kind="ExternalOutput").ap()
    with tile.TileContext(nc) as tc:
        _emit(tc, projs, out_partial)
    nc.compile()
    return nc


_NC_CACHE = None


def _get_nc():
    global _NC_CACHE
    if _NC_CACHE is None:
        _NC_CACHE = build()
    return _NC_CACHE


def make_in_maps(proj_1, proj_2):
    z = np.concatenate([np.asarray(proj_1, dtype=np.float32),
                        np.asarray(proj_2, dtype=np.float32)], axis=0)
    return [{"projs": np.ascontiguousarray(z[_RPC * c:_RPC * (c + 1)])}
            for c in range(_NCORES)]


def kernel(proj_1, proj_2):
    from concourse import bass_utils

    nc = _get_nc()
    in_maps = make_in_maps(proj_1, proj_2)
    r = bass_utils.run_bass_kernel_spmd(nc, in_maps,
                                        core_ids=list(range(_NCORES)))
    total = sum(float(res["partial"][0, 0]) for res in r.results)
    return np.float32(total / _N2)


# revision 8
# speedup vs baseline: 1.0533x; 1.0533x over previous
"""NT-Xent / contrastive loss on 8 Trainium2 NeuronCores.

Reference computation (B=4096, D=512, temp=0.1):
    z   = l2norm(concat(proj_1, proj_2))          # [8192, 512]
    cos = (z @ z.T) / temp                        # [8192, 8192]
    pos[r]  = cos[r, (r + 4096) % 8192]
    lse[r]  = logsumexp(cos[r, :] with cos[r, r] masked out)
    loss    = mean(lse - pos)

Sharding: similarity-matrix rows, 1024 per core; inputs host-rotated by
core*1024 rows so the SPMD program is uniform (self-diagonal at local
col == row, positive at local col == row + 4096).

v2.5 design (vs 238us bf16 baseline, 169us fp8 v2):
  - Each core normalizes/transposes ONLY its own 1024 rows: dense f32
    pair loads, square+rowsum (DVE stt), fast-rsqrt (DVE int magic + 2
    Newton steps), normalize+bf16 downcast (ts_mul by per-partition
    16/||x||), PE transpose vs bf16 identity, PSUM->SBUF fp8 e4m3
    evacuation.  The fp8 transposed block zq.T = (16*l2norm(x)).T
    [128, 4, 1024] is AllGathered across the 8 cores via HBM (512KB per
    core), then 7 dynamic-offset DMAs (partition_id register) re-rotate
    the gathered ranks into this core's local column order.  This
    removes 7/8 of the phase-1 vector work and input DMA vs v2.
  - GEMM in fp8 with MatmulPerfMode.DoubleRow (2 K-subtiles per
    instruction) = 2x bf16 PE throughput; psum = 256*cos, folded out
    via the Exp scale 10/256.  The J=0 column group (own rows, needs no
    collective) is emitted first so the PE works during the AllGather.
  - Row logsumexp: ScalarE Exp (in-place PSUM, accum_out row sums) for
    most J-groups; a tuned subset uses a DVE Schraudolph exp (int32
    bits = A*x + B, zero-mean constant) + reduce_sum to balance the
    exp-bound ScalarE against the otherwise-idle DVE.  Self/positive
    diagonals are pulled from raw PSUM (multiply-by-identity with
    accum) before the in-place Exp; the self term is subtracted via the
    identical ScalarE Exp pathway so it cancels exactly.
Host adds the 8 partial sums and divides by 8192.
"""

import sys

import numpy as np

if "/opt/trn_rl_repo" not in sys.path:
    sys.path.insert(0, "/opt/trn_rl_repo")

_B = 4096
_D = 512
_N2 = 2 * _B            # 8192 rows of the similarity matrix
_NCORES = 8
_RPC = _N2 // _NCORES   # 1024 rows per core
_INV_TEMP = 10.0
_FSC = 16.0             # fp8 scale: zq = 16*z, psum = 256*cos
_ESC = _INV_TEMP / (_FSC * _FSC)   # Exp scale applied to psum

_GRP = 8                # own row tiles per core
_NM = _RPC // 128       # 8 output row blocks
_NG = _N2 // 1024       # 8 column groups (one per source rank)
_NK = _D // 128         # 4 contraction chunks (2 DoubleRow pairs)

_MAGIC1 = 0x5F3759E0    # fast inverse sqrt magic + 1 (M - x == (M+1) + ~x)

# Schraudolph exp on psum: float32 bits = A*x + B (zero-mean constant)
_SCH_C = 0.05640058203329989
_SCH_A = float((2.0 ** 23) / np.log(2.0) * _ESC)
_SCH_B = float(127.0 * 2 ** 23 - _SCH_C * 2 ** 23)


def _sch_group(m, g):
    # J-groups routed to the DVE Schraudolph exp, spread across g so DVE
    # and ScalarE stay concurrently busy; groups 0 (self diag) and 4
    # (positive diag) must stay on ScalarE.
    if g in (0, 4):
        return False
    return (m + g) % 3 == 0 or (g in (1, 5) and (m + g) % 3 == 1)


def _emit(tc, projs, out_partial):
    import concourse.bass as bass  # noqa: F401
    from concourse.bass import ds
    from concourse import mybir

    nc = tc.nc
    f32 = mybir.dt.float32
    bf16 = mybir.dt.bfloat16
    fp8 = mybir.dt.float8e4
    i32 = mybir.dt.int32
    Alu = mybir.AluOpType
    Act = mybir.ActivationFunctionType
    DR = mybir.MatmulPerfMode.DoubleRow

    from contextlib import ExitStack
    ctx = ExitStack()
    pool = ctx.enter_context(tc.tile_pool(name="work", bufs=1))
    pers = ctx.enter_context(tc.tile_pool(name="pers", bufs=1))
    pspool = ctx.enter_context(tc.tile_pool(name="psum", bufs=1, space="PSUM"))
    dram = ctx.enter_context(tc.tile_pool(name="dram", bufs=1, space="DRAM"))

    # ---- constants ----
    ones = pers.tile([128, 128], f32, tag="ones")
    nc.vector.memset(ones[:], 1.0)
    ident = pers.tile([128, 128], f32, tag="ident")
    nc.gpsimd.affine_select(ident[:], ones[:], pattern=[[1, 128]],
                            compare_op=Alu.is_equal, fill=0.0,
                            base=0, channel_multiplier=-1)
    identb = pers.tile([128, 128], bf16, tag="identb")
    nc.vector.tensor_copy(identb[:], ident[:])

    # ---- persistent buffers ----
    # own transposed fp8 block: [p, k, own-row c] (c in 0..1023)
    zown = pers.tile([128, _NK * _RPC], fp8, tag="zown")
    zo3 = zown.rearrange("p (k c) -> p k c", k=_NK)
    # gathered/rotated transposed z, local column order [p, k, 0..8191]
    zt = pers.tile([128, _NK * _N2], fp8, tag="zt")
    zt3 = zt.rearrange("p (k c) -> p k c", k=_NK)
    sp_all = pers.tile([128, 2 * _NM], f32, tag="sp")    # self | pos diags
    se_all = pers.tile([128, _NM * _NG], f32, tag="se")  # per (m, J) sums

    pv = projs.rearrange("(u two p) d -> u p two d", two=2, p=128)

    # ---- phase 1: own 8 tiles only ----
    raw2s = []
    ss = pool.tile([128, _GRP], f32, tag="ss")
    for u in range(_GRP // 2):
        raw2 = pool.tile([128, 2, _D], f32, tag="raw", bufs=4, name=f"raw{u}")
        nc.sync.dma_start(raw2[:], pv[u])
        raw2s.append(raw2)
        for h in range(2):
            i = 2 * u + h
            sq = pool.tile([128, _D], bf16, tag="sq", bufs=2, name=f"sq{i}")
            nc.vector.scalar_tensor_tensor(
                out=sq[:], in0=raw2[:, h], scalar=1.0, in1=raw2[:, h],
                op0=Alu.mult, op1=Alu.mult, accum_out=ss[:, i:i + 1])

    # rn16 = 16/sqrt(max(ss,1e-24)): fast-rsqrt + 2 Newton steps
    ssc = pool.tile([128, _GRP], f32, tag="ssc")
    nc.vector.tensor_scalar_max(ssc[:], ss[:], 1e-24)
    ti = pool.tile([128, _GRP], i32, tag="ti")
    nc.vector.tensor_scalar(
        out=ti[:], in0=ssc[:].bitcast(i32), scalar1=1, scalar2=-1,
        op0=Alu.logical_shift_right, op1=Alu.bitwise_xor)
    rn = pool.tile([128, _GRP], f32, tag="rn")
    nc.vector.tensor_scalar(
        out=rn[:].bitcast(i32), in0=ti[:], scalar1=_MAGIC1, scalar2=None,
        op0=Alu.add)
    nt = pool.tile([128, _GRP], f32, tag="nt")
    for _ in range(2):
        nc.vector.tensor_tensor(out=nt[:], in0=rn[:], in1=rn[:], op=Alu.mult)
        nc.vector.tensor_tensor(out=nt[:], in0=nt[:], in1=ssc[:], op=Alu.mult)
        nc.vector.tensor_scalar(out=nt[:], in0=nt[:], scalar1=-0.5,
                                scalar2=1.5, op0=Alu.mult, op1=Alu.add)
        nc.vector.tensor_tensor(out=rn[:], in0=rn[:], in1=nt[:], op=Alu.mult)
    rn16 = pool.tile([128, _GRP], f32, tag="rn16")
    nc.vector.tensor_scalar(out=rn16[:], in0=rn[:], scalar1=_FSC,
                            scalar2=None, op0=Alu.mult)

    for i in range(_GRP):
        rawb = pool.tile([128, _D], bf16, tag="rawb", bufs=4, name=f"rawb{i}")
        nc.vector.tensor_scalar_mul(rawb[:], raw2s[i // 2][:, i % 2],
                                    rn16[:, i:i + 1])
        psT = pspool.tile([128, _D], f32, tag="psT", bufs=2, name=f"psT{i}")
        for d in range(_NK):
            nc.tensor.matmul(psT[:, d * 128:(d + 1) * 128],
                             rawb[:, d * 128:(d + 1) * 128],
                             identb[:], start=True, stop=True)
        dst = zo3[:, :, i * 128:(i + 1) * 128]
        srcT = psT[:].rearrange("p (k c) -> p k c", k=_NK)
        if i % 2 == 0:
            nc.vector.tensor_copy(dst, srcT)
        else:
            nc.scalar.activation(dst, srcT, Act.Copy, bias=0.0, scale=1.0)

    # ---- AllGather own fp8 block, then re-rotate into local order ----
    gin = dram.tile([128, _NK * _RPC], fp8)
    gout = nc.dram_tensor("gout_shared", [_NCORES * 128, _NK * _RPC], fp8,
                          addr_space="Shared").ap()
    nc.sync.dma_start(gin[:], zown[:])
    nc.gpsimd.collective_compute(
        "AllGather", Alu.bypass, replica_groups=[list(range(_NCORES))],
        ins=[gin[:].opt()], outs=[gout.opt()])
    go4 = gout.rearrange("(r p) (k c) -> r p k c", p=128, k=_NK)

    cid = nc.sync.partition_id()
    for g in range(1, _NG):
        src_r = nc.sync.alloc_register(f"srcr{g}")
        nc.sync.reg_alu(src_r, cid, g, Alu.add)
        nc.sync.reg_alu(src_r, src_r, _NCORES, Alu.mod)
        src = nc.sync.snap(src_r, min_val=0, max_val=_NCORES - 1)
        nc.sync.dma_start(zt3[:, :, g * _RPC:(g + 1) * _RPC],
                          go4[ds(src, 1), :, :, :])

    # ---- phase 2: DoubleRow fp8 GEMM + exp row sums ----
    def gemm_expcol(m, g):
        rhs3 = zo3 if g == 0 else zt3
        ps = pspool.tile([128, 1024], f32, tag="ps", bufs=3,
                         name=f"ps{m}_{g}")
        for c in range(2):
            jlo = g * 1024 + c * 512
            for k2 in range(2):
                nc.tensor.matmul(
                    ps[:, c * 512:(c + 1) * 512],
                    zo3[:, 2 * k2:2 * k2 + 2, m * 128:(m + 1) * 128],
                    rhs3[:, 2 * k2:2 * k2 + 2, jlo:jlo + 512],
                    start=(k2 == 0), stop=(k2 == 1), perf_mode=DR)
        off = m * 128
        if g == 0 or g == _NG // 2:
            col = m if g == 0 else _NM + m
            junk = pool.tile([128, 128], f32, tag="junk", bufs=2,
                             name=f"junk{m}_{g}")
            nc.vector.scalar_tensor_tensor(
                out=junk[:], in0=ps[:, off:off + 128], scalar=1.0,
                in1=ident[:], op0=Alu.mult, op1=Alu.mult,
                accum_out=sp_all[:, col:col + 1])
        secol = se_all[:, m * _NG + g:m * _NG + g + 1]
        if _sch_group(m, g):
            tmp = pool.tile([128, 1024], i32, tag="tmp", bufs=2,
                            name=f"tmp{m}_{g}")
            nc.vector.tensor_scalar(
                out=tmp[:], in0=ps[:], scalar1=_SCH_A, scalar2=_SCH_B,
                op0=Alu.mult, op1=Alu.add)
            nc.vector.reduce_sum(out=secol, in_=tmp[:].bitcast(f32),
                                 axis=mybir.AxisListType.X)
        else:
            nc.scalar.activation(ps[:], ps[:], Act.Exp, bias=0.0,
                                 scale=_ESC, accum_out=secol)

    # J=0 first (no collective dependency) so PE overlaps the AllGather
    for g in range(_NG):
        for m in range(_NM):
            gemm_expcol(m, g)

    # ---- phase 3: lse, loss, partial sum ----
    rs = pool.tile([128, _NM], f32, tag="rs")
    se4 = se_all.rearrange("p (m j) -> p m j", m=_NM)
    nc.vector.tensor_reduce(out=rs[:], in_=se4, axis=mybir.AxisListType.X,
                            op=Alu.add)
    sx = pool.tile([128, _NM], f32, tag="sx")
    nc.scalar.activation(sx[:], sp_all[:, 0:_NM], Act.Exp, bias=0.0,
                         scale=_ESC)
    nc.vector.tensor_sub(rs[:], rs[:], sx[:])
    lse = pool.tile([128, _NM], f32, tag="lse")
    nc.scalar.activation(lse[:], rs[:], Act.Ln, bias=0.0, scale=1.0)
    loss = pool.tile([128, _NM], f32, tag="loss")
    nc.vector.scalar_tensor_tensor(
        out=loss[:], in0=sp_all[:, _NM:2 * _NM], scalar=-_ESC,
        in1=lse[:], op0=Alu.mult, op1=Alu.add)
    lossv = pool.tile([128, 1], f32, tag="lossv")
    nc.vector.reduce_sum(out=lossv[:], in_=loss[:], axis=mybir.AxisListType.X)
    pf = pspool.tile([1, 1], f32, tag="psT", bufs=2)
    nc.tensor.matmul(pf[:], lossv[:], ones[:, 0:1], start=True, stop=True)
    res = pool.tile([1, 1], f32, tag="res")
    nc.vector.tensor_copy(res[:], pf[:])
    nc.sync.dma_start(out_partial[:, :], res[:])

    ctx.close()


def build():
    import concourse.tile as tile
    from concourse import bacc, mybir

    nc = bacc.Bacc("TRN2", target_bir_lowering=False, debug=False,
                   enable_asserts=True, num_devices=_NCORES)
    projs = nc.dram_tensor("projs", [_RPC, _D], mybir.dt.float32,
                           kind="ExternalInput").ap()
    out_partial = nc.dram_tensor("partial", [1, 1], mybir.dt.float32,
                                 kind="ExternalOutput").ap()
    with tile.TileContext(nc) as tc:
        _emit(tc, projs, out_partial)
    nc.compile()
    return nc


_NC_CACHE = None


def _get_nc():
    global _NC_CACHE
    if _NC_CACHE is None:
        _NC_CACHE = build()
    return _NC_CACHE


def make_in_maps(proj_1, proj_2):
    z = np.concatenate([np.asarray(proj_1, dtype=np.float32),
                        np.asarray(proj_2, dtype=np.float32)], axis=0)
    return [{"projs": np.ascontiguousarray(z[_RPC * c:_RPC * (c + 1)])}
            for c in range(_NCORES)]


def kernel(proj_1, proj_2):
    from concourse import bass_utils

    nc = _get_nc()
    in_maps = make_in_maps(proj_1, proj_2)
    r = bass_utils.run_bass_kernel_spmd(nc, in_maps,
                                        core_ids=list(range(_NCORES)))
    total = sum(float(res["partial"][0, 0]) for res in r.results)
    return np.float32(total / _N2)
